# revision 24
# baseline (speedup 1.0000x reference)
"""Trainium2 Bass kernel for nn_Attention_22600117911625.

Multi-head causal attention with interleaved RoPE:
  out = softmax(mask(RoPE(xWq^T) RoPE(xWk^T)^T / sqrt(128))) (xWv^T) Wo^T

Sharding over 8 NeuronCores: data-parallel over batch (2) x tensor-parallel
over 4 head-groups (4 heads each).  All matmuls in bf16 (same PE issue rate
as fp32r at N=512, half the DMA/SBUF), fp32 PSUM accumulation.

Per core, single fused pass with Q/K/V resident in SBUF (no DRAM spill):
  phase 1: Q^T/K^T (head-dim-major, de-interleave-permuted) + V projections
           from x^T; RoPE via swap-matmul + cos/sin tables; results written
           straight into SBUF-resident qt/kt/vt tiles (bf16).
  phase 2: j-outer, head-inner transposed flash attention: S^T chunk pairs,
           causally trimmed exp on ScalarE (bf16 out), row sums via a
           ones-matmul, PV in PSUM, 1/l on VectorE, normalized A^T (bf16)
           DMA'd to DRAM; one AllGather per j-block over the 4-core group.
  phase 3: interleaved one j-block behind phase 2: out^T columns for the
           core's 512 d_model rows from the gathered A^T.
Host side only reshapes/casts inputs and concatenates/transposes outputs.
"""
import math

import numpy as np

import concourse.bass as bass
import concourse.mybir as mybir
from concourse import bass2jax
from concourse.tile import TileContext
from concourse.vector_clock import ScopedClock

F32 = mybir.dt.float32
BF16 = mybir.dt.bfloat16
AF = mybir.ActivationFunctionType

B = 2
S = 4096
DM = 2048
H = 16
DH = 128
N_CORES = 8
GROUPS = 4          # tensor-parallel head groups
HL = H // GROUPS    # heads per core (4)
EL = HL * DH        # local head width (512)
SB = 512            # s-block width
NSB = S // SB       # 8
ECH = DM // 128     # 16 e-chunks
SCALE = 1.0 / math.sqrt(DH)
MASK_NEG = -3.0e8
REPLICA_GROUPS = [[0, 1, 2, 3], [4, 5, 6, 7]]

_wsplit_cnt = [0]


class TC(TileContext):
    """TileContext for a walrus build that allows only ONE semaphore wait per
    instruction: extra waits are split onto nofuse NOPs on the same engine."""

    def _lower_ordered_insts(self, ordered):
        for bb_name in list(ordered.keys()):
            new = []
            for inst in ordered[bb_name]:
                si = getattr(inst, "sync_info", None)
                if si is not None and len(si.on_wait) > 1:
                    waits = list(si.on_wait)
                    eng = getattr(inst, "engine", None)
                    if eng is not None:
                        for w in waits[:-1]:
                            _wsplit_cnt[0] += 1
                            new.append(mybir.InstNoOp(
                                name=f"wsplit{_wsplit_cnt[0]}",
                                sync_info=mybir.SyncInfo(on_wait=[w], on_update=[]),
                                bass_nofuse=True,
                                engine=eng,
                            ))
                        inst.sync_info = mybir.SyncInfo(
                            on_wait=[waits[-1]], on_update=list(si.on_update))
                new.append(inst)
            ordered[bb_name] = new
        super()._lower_ordered_insts(ordered)

    def _drain_and_barrier(self, tick_clock, wait_clock):
        probe = self.nc.sync.nop(nofuse=True, hint="drain_wait_probe")
        probe.ins.sync_info = mybir.SyncInfo(on_wait=[], on_update=[])
        wait_clock.add_sem_waits(probe.ins, ScopedClock({None: tick_clock.global_clock}))
        waits = list(probe.ins.sync_info.on_wait)
        probe.ins.sync_info = mybir.SyncInfo(on_wait=waits[:1], on_update=[])
        for w in waits[1:]:
            n = self.nc.sync.nop(nofuse=True, hint="drain_wait_split")
            n.ins.sync_info = mybir.SyncInfo(on_wait=[w], on_update=[])
        self.nc.sync.drain()
        self.nc.all_engine_barrier()
        popped = self.nc._tile_sem_poison_stack.pop()
        assert popped is self._sem_poison
        self.nc.clear_and_free_semaphores(list(self.sems.allocated().values()))
        self.nc.all_engine_barrier()


def build_nc():
    nc = bass.Bass()

    xT = nc.declare_dram_parameter("xT", [DM, S], BF16, isOutput=False)
    wq = nc.declare_dram_parameter("wq", [128, ECH * EL], BF16, isOutput=False)
    wk = nc.declare_dram_parameter("wk", [128, ECH * EL], BF16, isOutput=False)
    wv = nc.declare_dram_parameter("wv", [128, ECH * EL], BF16, isOutput=False)
    wo = nc.declare_dram_parameter("wo", [HL, 128, DM], BF16, isOutput=False)
    cosT = nc.declare_dram_parameter("cosT", [128, S], BF16, isOutput=False)
    sinT = nc.declare_dram_parameter("sinT", [128, S], BF16, isOutput=False)
    swapM = nc.declare_dram_parameter("swapM", [128, 128], BF16, isOutput=False)
    onesW = nc.declare_dram_parameter("onesW", [128, 128], BF16, isOutput=False)
    idM = nc.declare_dram_parameter("idM", [128, 128], BF16, isOutput=False)
    masks = nc.declare_dram_parameter("masks", [4, 128, SB], BF16, isOutput=False)
    # phase 3 is sharded over d_model: this core computes out^T rows for its
    # group's 512 d_model columns (selected host-side via the wo slice).
    outT = nc.declare_dram_parameter("outT", [EL, S], F32, isOutput=True)

    with TC(nc) as tc:
        with (
            tc.tile_pool(name="const", bufs=1) as constp,
            tc.tile_pool(name="qkv", bufs=1) as qkvp,
            tc.tile_pool(name="dram", bufs=1, space="DRAM") as dram,
        ):
            ones_sb = constp.tile([128, 128], BF16)
            nc.sync.dma_start(out=ones_sb[:], in_=onesW[:])
            swap_sb = constp.tile([128, 128], BF16)
            nc.sync.dma_start(out=swap_sb[:], in_=swapM[:])
            id_sb = constp.tile([128, 128], BF16)
            nc.sync.dma_start(out=id_sb[:], in_=idM[:])

            # SBUF-resident projections (bf16): 12 x 8KiB/partition
            qt = [qkvp.tile([128, S], BF16, name=f"qt{h}") for h in range(HL)]
            kt = [qkvp.tile([128, S], BF16, name=f"kt{h}") for h in range(HL)]
            vt = qkvp.tile([128, HL * S], BF16, name="vt")

            # per-j-block gather buffers; the last block gathers per head so
            # the kernel tail only waits on one head's worth of collective
            aT_l = [dram.tile([128, HL * SB], BF16, name=f"aT_l{j}")
                    for j in range(NSB - 1)]
            aT_f = [dram.tile([GROUPS * 128, HL * SB], BF16, name=f"aT_f{j}")
                    for j in range(NSB - 1)]
            aT_l7 = [dram.tile([128, SB], BF16, name=f"aT_l7h{h}")
                     for h in range(HL)]
            aT_f7 = [dram.tile([GROUPS * 128, SB], BF16, name=f"aT_f7h{h}")
                     for h in range(HL)]

            _phase1(nc, tc, xT, wq, wk, wv, cosT, sinT, swap_sb,
                    qt, kt, vt)
            _phase23(nc, tc, ones_sb, id_sb, masks, qt, kt, vt,
                     aT_l, aT_f, aT_l7, aT_f7, wo, outT)
    return nc


def _phase1(nc, tc, xT, wq, wk, wv, cosT, sinT, swap_sb, qt, kt, vt):
    with (
        tc.tile_pool(name="p1w", bufs=1) as wpool,
        tc.tile_pool(name="p1x", bufs=2) as xpool,
        tc.tile_pool(name="p1st", bufs=3) as stage,
        tc.tile_pool(name="p1ps", bufs=2, space="PSUM") as pspool,
        tc.tile_pool(name="p1psv", bufs=2, space="PSUM") as psvpool,
        tc.tile_pool(name="p1pssw", bufs=2, space="PSUM") as psswap,
    ):
        # weight/table loads go on the ACT HWDGE queue (idle at start, no
        # waits) so they land in parallel with the x-tile loads on SP
        wq_sb = wpool.tile([128, ECH * EL], BF16)
        wk_sb = wpool.tile([128, ECH * EL], BF16)
        wv_sb = wpool.tile([128, ECH * EL], BF16)
        for part in range(4):
            sl = slice(part * 4 * EL, (part + 1) * 4 * EL)
            nc.scalar.dma_start(out=wq_sb[:, sl], in_=wq[:, sl])
        for part in range(4):
            sl = slice(part * 4 * EL, (part + 1) * 4 * EL)
            nc.scalar.dma_start(out=wk_sb[:, sl], in_=wk[:, sl])
            nc.scalar.dma_start(out=wv_sb[:, sl], in_=wv[:, sl])
        cos_sb = wpool.tile([128, S], BF16)
        nc.scalar.dma_start(out=cos_sb[:], in_=cosT[:])
        sin_sb = wpool.tile([128, S], BF16)
        nc.scalar.dma_start(out=sin_sb[:], in_=sinT[:])

        xT_r = xT[:].rearrange("(ec p) s -> p ec s", p=128)
        for sb in range(NSB):
            xt = xpool.tile([128, ECH * SB], BF16, tag="xt")
            for xh in range(2):
                nc.sync.dma_start(
                    out=xt[:, xh * 8 * SB:(xh + 1) * 8 * SB].rearrange(
                        "p (ec s) -> p ec s", ec=ECH // 2),
                    in_=xT_r[:, xh * 8:(xh + 1) * 8,
                             sb * SB:(sb + 1) * SB])

            # Q^T and K^T head-tiles, accumulated over e-chunks, then RoPE
            for wsb, dst in ((wq_sb, qt), (wk_sb, kt)):
                for h in range(HL):
                    ps = pspool.tile([128, SB], F32, tag="proj")
                    for ec in range(ECH):
                        nc.tensor.matmul(
                            ps[:],
                            wsb[:, ec * EL + h * 128: ec * EL + (h + 1) * 128],
                            xt[:, ec * SB:(ec + 1) * SB],
                            start=(ec == 0), stop=(ec == ECH - 1))
                    raw = stage.tile([128, SB], BF16, tag="raw")
                    nc.scalar.copy(raw[:], ps[:])
                    ps_sw = psswap.tile([128, SB], F32, tag="swap")
                    nc.tensor.matmul(ps_sw[:], swap_sb[:], raw[:],
                                     start=True, stop=True)
                    t1 = stage.tile([128, SB], BF16, tag="t1")
                    nc.vector.tensor_mul(t1[:], ps[:],
                                         cos_sb[:, sb * SB:(sb + 1) * SB])
                    t2 = stage.tile([128, SB], BF16, tag="t2")
                    nc.vector.tensor_mul(t2[:], ps_sw[:],
                                         sin_sb[:, sb * SB:(sb + 1) * SB])
                    nc.vector.tensor_add(
                        dst[h][:, sb * SB:(sb + 1) * SB], t1[:], t2[:])

            # V natural layout: lhsT = x^T chunk slice (stationary), rhs = wv
            for st in range(4):
                psv = psvpool.tile([128, EL], F32, tag="projv")
                for ec in range(ECH):
                    nc.tensor.matmul(
                        psv[:],
                        xt[:, ec * SB + st * 128: ec * SB + (st + 1) * 128],
                        wv_sb[:, ec * EL:(ec + 1) * EL],
                        start=(ec == 0), stop=(ec == ECH - 1))
                # scatter per-head columns into vt in one strided copy
                # (psv free dim is h*128+d, vt free dim is h*S+s_chunk*128+d)
                stg = sb * 4 + st
                nc.scalar.copy(
                    vt[:].rearrange("p (h s) -> p h s", h=HL)
                    [:, :, stg * 128:(stg + 1) * 128],
                    psv[:].rearrange("p (h d) -> p h d", h=HL))


def _phase23(nc, tc, ones_sb, id_sb, masks, qt, kt, vt,
             aT_l, aT_f, aT_l7, aT_f7, wo, outT):
    with (
        tc.tile_pool(name="p2m", bufs=1) as mpool,
        tc.tile_pool(name="p2pt", bufs=3) as ptpool,
        tc.tile_pool(name="p2st", bufs=3) as stage,
        tc.tile_pool(name="p2pss", bufs=2, space="PSUM") as pss,
        tc.tile_pool(name="p2psa", bufs=2, space="PSUM") as psa,
        tc.tile_pool(name="p2psl", bufs=1, space="PSUM") as psl,
        tc.tile_pool(name="p3w", bufs=1) as wopool,
        tc.tile_pool(name="p3a", bufs=12) as apool,
        tc.tile_pool(name="p3st", bufs=2) as stage3,
        tc.tile_pool(name="p3ps", bufs=1, space="PSUM") as pso,
    ):
        masks_sb = mpool.tile([128, 4 * SB], BF16)
        for p in range(4):
            nc.sync.dma_start(out=masks_sb[:, p * SB:(p + 1) * SB],
                              in_=masks[p])
        wot = []
        for dml in range(HL):
            t = wopool.tile([128, DM], BF16, tag=f"wo{dml}")
            nc.sync.dma_start(out=t[:], in_=wo[dml])
            wot.append(t)

        def atg_load(b, gate=None):
            """Prefetch the gathered A^T of block b into SBUF (4 tiles).

            `gate` is an SBUF tile from the flash stream: a tiny copy of it
            into each destination tile makes the load (and everything that
            consumes it) order after that point of the flash stream in BOTH
            the scheduler's model and on hardware — without it the scheduler
            hoists the dependent out-projection matmuls into earlier exp-wait
            bubbles, where they stall the in-order PE queue on the (slow)
            AllGather."""
            atg = []
            for r in range(GROUPS):
                t = apool.tile([128, HL * SB], BF16, tag="atf")
                if gate is not None:
                    nc.vector.tensor_copy(t[:, :16], gate[:, :16])
                if b < NSB - 1:
                    nc.gpsimd.dma_start(
                        out=t[:], in_=aT_f[b][r * 128:(r + 1) * 128, :])
                else:
                    for h in range(HL):
                        nc.gpsimd.dma_start(
                            out=t[:, h * SB:(h + 1) * SB],
                            in_=aT_f7[h][r * 128:(r + 1) * 128, :])
                atg.append(t)
            return atg

        def out_steps(b, atg):
            """Generator of phase-3 emission steps for block b."""
            for dml in range(HL):
                ps = pso.tile([128, SB], F32, tag="o")
                for i in range(ECH):
                    h, r = divmod(i, GROUPS)   # head-major: head 3 last
                    nc.tensor.matmul(
                        ps[:],
                        wot[dml][:, (r * HL + h) * 128:(r * HL + h + 1) * 128],
                        atg[r][:, h * SB:(h + 1) * SB],
                        start=(i == 0), stop=(i == ECH - 1))
                    yield
                osb = stage3.tile([128, SB], F32, tag="osb")
                nc.vector.tensor_copy(osb[:], ps[:])
                nc.sync.dma_start(
                    out=outT[dml * 128:(dml + 1) * 128,
                             b * SB:(b + 1) * SB],
                    in_=osb[:])
                yield

        atgs = {}
        pending = []

        def drain(n):
            if len(pending) > 1:
                n += 1
            done = 0
            while pending and done < n:
                try:
                    next(pending[0])
                    done += 1
                except StopIteration:
                    pending.pop(0)

        last = NSB - 1
        for j in range(NSB):
            for h in range(HL):
                at = _flash_block(nc, ones_sb, id_sb, masks_sb, qt, kt, vt,
                                  ptpool, stage, pss, psa, psl,
                                  aT_l, aT_l7, j, h, drain)
                if j == last:
                    nc.gpsimd.collective_compute(
                        "AllGather", mybir.AluOpType.bypass,
                        replica_groups=REPLICA_GROUPS,
                        ins=[aT_l7[h][:]], outs=[aT_f7[h][:]])
                # gated prefetch + interleave: block b = j-3 starts flowing
                # right after flash(j) h0; its AllGather finished during
                # flash(j-1).  At j == last, h == 1/2 pick up blocks 5 and 6.
                if h == 0 and j >= 3:
                    b = j - 3
                    atgs[b] = atg_load(b, gate=at)
                    pending.append(out_steps(b, atgs[b]))
                elif j == last and h == 1:
                    atgs[5] = atg_load(5, gate=at)
                    pending.append(out_steps(5, atgs[5]))
                elif j == last and h == 2:
                    atgs[6] = atg_load(6, gate=at)
            if j < last:
                nc.gpsimd.collective_compute(
                    "AllGather", mybir.AluOpType.bypass,
                    replica_groups=REPLICA_GROUPS,
                    ins=[aT_l[j][:]], outs=[aT_f[j][:]])
        # tail: remaining interleave leftovers, then blocks 6 and 7
        drain(10**9)
        for _ in out_steps(6, atgs[6]):
            pass
        atgs[last] = atg_load(last)
        for _ in out_steps(last, atgs[last]):
            pass


def _flash_block(nc, ones_sb, id_sb, masks_sb, qt, kt, vt, ptpool, stage,
                 pss, psa, psl, aT_l, aT_l7, j, h, drain):
    ps_a = psa.tile([128, SB], F32, tag="a")
    ps_l = psl.tile([128, SB], F32, tag="l")
    nk = 4 * j + 4
    npair = nk // 2
    for pr in range(npair):
        # two k-chunks share one 1024-wide PSUM tile so the exp runs once
        # per pair (amortizes ACT fixed overhead)
        ps_s = pss.tile([128, 2 * SB], F32, tag="s")
        pt = ptpool.tile([128, 2 * SB], BF16, tag="pt")
        offs = []
        for half in range(2):
            kc = 2 * pr + half
            # causal: columns below p*128 of a diagonal chunk are fully
            # masked; skip them in every consumer
            off = max(0, (kc - 4 * j) * 128) if kc >= 4 * j else 0
            offs.append(off)
            sl = slice(half * SB + off, (half + 1) * SB)
            diag = kc >= 4 * j
            nc.tensor.matmul(ps_s[:, sl],
                             kt[h][:, kc * 128:(kc + 1) * 128],
                             qt[h][:, j * SB + off:(j + 1) * SB],
                             start=True, stop=not diag)
            if diag:
                # add the causal mask on the PE: ps_s += I^T @ mask
                p = kc - 4 * j
                nc.tensor.matmul(
                    ps_s[:, sl], id_sb[:],
                    masks_sb[:, p * SB + off:(p + 1) * SB],
                    start=False, stop=True)
        nc.scalar.activation(pt[:, offs[0]:], ps_s[:, offs[0]:],
                             AF.Exp, scale=SCALE)
        for half in range(2):
            kc = 2 * pr + half
            off = offs[half]
            sl = slice(half * SB + off, (half + 1) * SB)
            osl = slice(off, SB)
            nc.tensor.matmul(ps_l[:, osl], ones_sb[:], pt[:, sl],
                             start=(kc == 0), stop=(kc == nk - 1))
            nc.tensor.matmul(ps_a[:, osl],
                             vt[:, h * S + kc * 128:h * S + (kc + 1) * 128],
                             pt[:, sl],
                             start=(kc == 0), stop=(kc == nk - 1))
        drain(2)
    lnl = stage.tile([128, SB], F32, tag="lnl")
    nc.scalar.activation(lnl[:], ps_l[:], AF.Ln)
    linv = stage.tile([128, SB], F32, tag="linv")
    nc.scalar.activation(linv[:], lnl[:], AF.Exp, scale=-1.0)
    at = stage.tile([128, SB], BF16, tag="at")
    nc.vector.tensor_mul(at[:], ps_a[:], linv[:])
    if j < NSB - 1:
        nc.sync.dma_start(out=aT_l[j][:, h * SB:(h + 1) * SB], in_=at[:])
    else:
        nc.sync.dma_start(out=aT_l7[h][:], in_=at[:])
    return at


def _host_prep(x, Wq, Wk, Wv, Wo):
    import ml_dtypes
    bf16 = ml_dtypes.bfloat16
    perm = np.concatenate([np.arange(0, DH, 2), np.arange(1, DH, 2)])  # evens then odds
    rowperm = np.concatenate([h * DH + perm for h in range(HL)])

    def tile_w(Wg):  # (EL, DM) -> (128, ECH*EL): [p, ec*EL+m] = Wg[m, ec*128+p]
        return np.ascontiguousarray(
            Wg.reshape(EL, ECH, 128).transpose(2, 1, 0).reshape(128, ECH * EL)
        ).astype(bf16)

    inv_freq = (1.0 / (10000.0 ** (np.arange(0, DH, 2) / DH))).astype(np.float64)
    pos = np.arange(S, dtype=np.float64)
    freqs = np.outer(inv_freq, pos)  # (64, S)
    cosT = np.concatenate([np.cos(freqs), np.cos(freqs)], 0).astype(bf16)
    sinT = np.concatenate([-np.sin(freqs), np.sin(freqs)], 0).astype(bf16)

    swap = np.zeros((128, 128), np.float32)
    for m in range(128):
        swap[(m + 64) % 128, m] = 1.0
    onesW = np.ones((128, 128), np.float32)
    masks = np.zeros((4, 128, SB), np.float32)
    ki = np.arange(128)[:, None]
    qi = np.arange(SB)[None, :]
    for p in range(4):
        masks[p] = np.where(qi >= ki + p * 128, 0.0, MASK_NEG)

    in_maps = []
    for c in range(N_CORES):
        b, g = divmod(c, GROUPS)
        sl = slice(g * EL, (g + 1) * EL)
        # wo[dml, p, ec*128+m] = Wo[g*EL + dml*128 + m, ec*128 + p]
        wo_t = np.ascontiguousarray(
            Wo[sl].reshape(HL, 128, ECH, 128).transpose(0, 3, 2, 1)
            .reshape(HL, 128, DM)).astype(bf16)
        in_maps.append({
            "xT": np.ascontiguousarray(x[b].T).astype(bf16),
            "wq": tile_w(Wq[sl][rowperm]),
            "wk": tile_w(Wk[sl][rowperm]),
            "wv": tile_w(Wv[sl]),
            "wo": wo_t,
            "cosT": cosT,
            "sinT": sinT,
            "swapM": swap.astype(bf16),
            "onesW": onesW.astype(bf16),
            "idM": np.eye(128, dtype=np.float32).astype(bf16),
            "masks": masks.astype(bf16),
        })
    return in_maps


def kernel(x, Wq, Wk, Wv, Wo):
    in_maps = _host_prep(x, Wq, Wk, Wv, Wo)
    nc = build_nc()
    res = bass2jax.run_bass_via_pjrt(nc, in_maps, n_cores=N_CORES)
    out = np.empty((B, S, DM), np.float32)
    for c in range(N_CORES):
        b, g = divmod(c, GROUPS)
        out[b, :, g * EL:(g + 1) * EL] = res[c]["outT"].T
    return out


if __name__ == "__main__":
    rng = np.random.default_rng(0)
    x = rng.standard_normal((B, S, DM)).astype(np.float32)
    Wq = (rng.standard_normal((H * DH, DM)) * 0.02).astype(np.float32)
    Wk = (rng.standard_normal((H * DH, DM)) * 0.02).astype(np.float32)
    Wv = (rng.standard_normal((H * DH, DM)) * 0.02).astype(np.float32)
    Wo = (rng.standard_normal((DM, H * DH)) * 0.02).astype(np.float32)
    out = kernel(x, Wq, Wk, Wv, Wo)
    print(out.shape, out.dtype)


# revision 26
# speedup vs baseline: 1.0722x; 1.0722x over previous
"""Trainium2 Bass kernel for nn_Attention_22600117911625.

Multi-head causal attention with interleaved RoPE:
  out = softmax(mask(RoPE(xWq^T) RoPE(xWk^T)^T / sqrt(128))) (xWv^T) Wo^T

Sharding over 8 NeuronCores: data-parallel over batch (2) x tensor-parallel
over 4 head-groups (4 heads each).  All matmuls in bf16 (same PE issue rate
as fp32r at N=512, half the DMA/SBUF), fp32 PSUM accumulation.

Per core, single fused pass with Q/K/V resident in SBUF (no DRAM spill):
  phase 1: Q^T/K^T (head-dim-major, de-interleave-permuted) + V projections
           from x^T; RoPE via swap-matmul + cos/sin tables; results written
           straight into SBUF-resident qt/kt/vt tiles (bf16).
  phase 2: j-outer, head-inner transposed flash attention: S^T chunk pairs,
           causally trimmed exp on ScalarE (bf16 out), row sums via a
           ones-matmul, PV in PSUM, 1/l on VectorE, normalized A^T (bf16)
           DMA'd to DRAM; one AllGather per j-block over the 4-core group.
  phase 3: interleaved one j-block behind phase 2: out^T columns for the
           core's 512 d_model rows from the gathered A^T.
Host side only reshapes/casts inputs and concatenates/transposes outputs.
"""
import math

import numpy as np

import concourse.bass as bass
import concourse.mybir as mybir
from concourse import bass2jax
from concourse.tile import TileContext
from concourse.vector_clock import ScopedClock

F32 = mybir.dt.float32
BF16 = mybir.dt.bfloat16
AF = mybir.ActivationFunctionType

B = 2
S = 4096
DM = 2048
H = 16
DH = 128
N_CORES = 8
GROUPS = 4          # tensor-parallel head groups
HL = H // GROUPS    # heads per core (4)
EL = HL * DH        # local head width (512)
SB = 512            # s-block width
NSB = S // SB       # 8
ECH = DM // 128     # 16 e-chunks
SCALE = 1.0 / math.sqrt(DH)
MASK_NEG = -3.0e8
REPLICA_GROUPS = [[0, 1, 2, 3], [4, 5, 6, 7]]

_wsplit_cnt = [0]


class TC(TileContext):
    """TileContext for a walrus build that allows only ONE semaphore wait per
    instruction: extra waits are split onto nofuse NOPs on the same engine."""

    def _lower_ordered_insts(self, ordered):
        for bb_name in list(ordered.keys()):
            new = []
            for inst in ordered[bb_name]:
                si = getattr(inst, "sync_info", None)
                if si is not None and len(si.on_wait) > 1:
                    waits = list(si.on_wait)
                    eng = getattr(inst, "engine", None)
                    if eng is not None:
                        for w in waits[:-1]:
                            _wsplit_cnt[0] += 1
                            new.append(mybir.InstNoOp(
                                name=f"wsplit{_wsplit_cnt[0]}",
                                sync_info=mybir.SyncInfo(on_wait=[w], on_update=[]),
                                bass_nofuse=True,
                                engine=eng,
                            ))
                        inst.sync_info = mybir.SyncInfo(
                            on_wait=[waits[-1]], on_update=list(si.on_update))
                new.append(inst)
            ordered[bb_name] = new
        super()._lower_ordered_insts(ordered)

    def _drain_and_barrier(self, tick_clock, wait_clock):
        probe = self.nc.sync.nop(nofuse=True, hint="drain_wait_probe")
        probe.ins.sync_info = mybir.SyncInfo(on_wait=[], on_update=[])
        wait_clock.add_sem_waits(probe.ins, ScopedClock({None: tick_clock.global_clock}))
        waits = list(probe.ins.sync_info.on_wait)
        probe.ins.sync_info = mybir.SyncInfo(on_wait=waits[:1], on_update=[])
        for w in waits[1:]:
            n = self.nc.sync.nop(nofuse=True, hint="drain_wait_split")
            n.ins.sync_info = mybir.SyncInfo(on_wait=[w], on_update=[])
        self.nc.sync.drain()
        self.nc.all_engine_barrier()
        popped = self.nc._tile_sem_poison_stack.pop()
        assert popped is self._sem_poison
        self.nc.clear_and_free_semaphores(list(self.sems.allocated().values()))
        self.nc.all_engine_barrier()


def build_nc():
    nc = bass.Bass()

    xT = nc.declare_dram_parameter("xT", [DM, S], BF16, isOutput=False)
    wq = nc.declare_dram_parameter("wq", [128, ECH * EL], BF16, isOutput=False)
    wk = nc.declare_dram_parameter("wk", [128, ECH * EL], BF16, isOutput=False)
    wv = nc.declare_dram_parameter("wv", [128, ECH * EL], BF16, isOutput=False)
    wo = nc.declare_dram_parameter("wo", [HL, 128, DM], BF16, isOutput=False)
    cosT = nc.declare_dram_parameter("cosT", [128, S], BF16, isOutput=False)
    sinT = nc.declare_dram_parameter("sinT", [128, S], BF16, isOutput=False)
    swapM = nc.declare_dram_parameter("swapM", [128, 128], BF16, isOutput=False)
    onesW = nc.declare_dram_parameter("onesW", [128, 128], BF16, isOutput=False)
    idM = nc.declare_dram_parameter("idM", [128, 128], BF16, isOutput=False)
    masks = nc.declare_dram_parameter("masks", [4, 128, SB], BF16, isOutput=False)
    # phase 3 is sharded over d_model: this core computes out^T rows for its
    # group's 512 d_model columns (selected host-side via the wo slice).
    outT = nc.declare_dram_parameter("outT", [EL, S], F32, isOutput=True)

    with TC(nc) as tc:
        with (
            tc.tile_pool(name="const", bufs=1) as constp,
            tc.tile_pool(name="qkv", bufs=1) as qkvp,
            tc.tile_pool(name="dram", bufs=1, space="DRAM") as dram,
        ):
            ones_sb = constp.tile([128, 128], BF16)
            nc.sync.dma_start(out=ones_sb[:], in_=onesW[:])
            swap_sb = constp.tile([128, 128], BF16)
            nc.sync.dma_start(out=swap_sb[:], in_=swapM[:])
            id_sb = constp.tile([128, 128], BF16)
            nc.sync.dma_start(out=id_sb[:], in_=idM[:])

            # SBUF-resident projections (bf16): 12 x 8KiB/partition
            qt = [qkvp.tile([128, S], BF16, name=f"qt{h}") for h in range(HL)]
            kt = [qkvp.tile([128, S], BF16, name=f"kt{h}") for h in range(HL)]
            vt = qkvp.tile([128, HL * S], BF16, name="vt")

            # per-j-block gather buffers; the last block gathers per head so
            # the kernel tail only waits on one head's worth of collective
            aT_l = [dram.tile([128, HL * SB], BF16, name=f"aT_l{j}")
                    for j in range(NSB - 1)]
            aT_f = [dram.tile([GROUPS * 128, HL * SB], BF16, name=f"aT_f{j}")
                    for j in range(NSB - 1)]
            aT_l7 = [dram.tile([128, SB], BF16, name=f"aT_l7h{h}")
                     for h in range(HL)]
            aT_f7 = [dram.tile([GROUPS * 128, SB], BF16, name=f"aT_f7h{h}")
                     for h in range(HL)]

            _phase1(nc, tc, xT, wq, wk, wv, cosT, sinT, swap_sb,
                    qt, kt, vt)
            _phase23(nc, tc, ones_sb, id_sb, masks, qt, kt, vt,
                     aT_l, aT_f, aT_l7, aT_f7, wo, outT)
    return nc


def _phase1(nc, tc, xT, wq, wk, wv, cosT, sinT, swap_sb, qt, kt, vt):
    with (
        tc.tile_pool(name="p1w", bufs=1) as wpool,
        tc.tile_pool(name="p1x", bufs=2) as xpool,
        tc.tile_pool(name="p1st", bufs=3) as stage,
        tc.tile_pool(name="p1ps", bufs=2, space="PSUM") as pspool,
        tc.tile_pool(name="p1psv", bufs=2, space="PSUM") as psvpool,
        tc.tile_pool(name="p1pssw", bufs=2, space="PSUM") as psswap,
    ):
        # weight/table loads go on the ACT HWDGE queue (idle at start, no
        # waits) so they land in parallel with the x-tile loads on SP
        wq_sb = wpool.tile([128, ECH * EL], BF16)
        wk_sb = wpool.tile([128, ECH * EL], BF16)
        wv_sb = wpool.tile([128, ECH * EL], BF16)
        for part in range(4):
            sl = slice(part * 4 * EL, (part + 1) * 4 * EL)
            nc.scalar.dma_start(out=wq_sb[:, sl], in_=wq[:, sl])
        for part in range(4):
            sl = slice(part * 4 * EL, (part + 1) * 4 * EL)
            nc.scalar.dma_start(out=wk_sb[:, sl], in_=wk[:, sl])
            nc.scalar.dma_start(out=wv_sb[:, sl], in_=wv[:, sl])
        cos_sb = wpool.tile([128, S], BF16)
        nc.scalar.dma_start(out=cos_sb[:], in_=cosT[:])
        sin_sb = wpool.tile([128, S], BF16)
        nc.scalar.dma_start(out=sin_sb[:], in_=sinT[:])

        xT_r = xT[:].rearrange("(ec p) s -> p ec s", p=128)
        for sb in range(NSB):
            xt = xpool.tile([128, ECH * SB], BF16, tag="xt")
            for xh in range(2):
                nc.sync.dma_start(
                    out=xt[:, xh * 8 * SB:(xh + 1) * 8 * SB].rearrange(
                        "p (ec s) -> p ec s", ec=ECH // 2),
                    in_=xT_r[:, xh * 8:(xh + 1) * 8,
                             sb * SB:(sb + 1) * SB])

            # Q^T and K^T head-tiles, accumulated over e-chunks, then RoPE
            for wsb, dst in ((wq_sb, qt), (wk_sb, kt)):
                for h in range(HL):
                    ps = pspool.tile([128, SB], F32, tag="proj")
                    for ec in range(ECH):
                        nc.tensor.matmul(
                            ps[:],
                            wsb[:, ec * EL + h * 128: ec * EL + (h + 1) * 128],
                            xt[:, ec * SB:(ec + 1) * SB],
                            start=(ec == 0), stop=(ec == ECH - 1))
                    raw = stage.tile([128, SB], BF16, tag="raw")
                    nc.scalar.copy(raw[:], ps[:])
                    ps_sw = psswap.tile([128, SB], F32, tag="swap")
                    nc.tensor.matmul(ps_sw[:], swap_sb[:], raw[:],
                                     start=True, stop=True)
                    t1 = stage.tile([128, SB], BF16, tag="t1")
                    nc.vector.tensor_mul(t1[:], ps[:],
                                         cos_sb[:, sb * SB:(sb + 1) * SB])
                    t2 = stage.tile([128, SB], BF16, tag="t2")
                    nc.vector.tensor_mul(t2[:], ps_sw[:],
                                         sin_sb[:, sb * SB:(sb + 1) * SB])
                    nc.vector.tensor_add(
                        dst[h][:, sb * SB:(sb + 1) * SB], t1[:], t2[:])

            # V natural layout: lhsT = x^T chunk slice (stationary), rhs = wv
            for st in range(4):
                psv = psvpool.tile([128, EL], F32, tag="projv")
                for ec in range(ECH):
                    nc.tensor.matmul(
                        psv[:],
                        xt[:, ec * SB + st * 128: ec * SB + (st + 1) * 128],
                        wv_sb[:, ec * EL:(ec + 1) * EL],
                        start=(ec == 0), stop=(ec == ECH - 1))
                # scatter per-head columns into vt in one strided copy
                # (psv free dim is h*128+d, vt free dim is h*S+s_chunk*128+d)
                stg = sb * 4 + st
                nc.scalar.copy(
                    vt[:].rearrange("p (h s) -> p h s", h=HL)
                    [:, :, stg * 128:(stg + 1) * 128],
                    psv[:].rearrange("p (h d) -> p h d", h=HL))


def _phase23(nc, tc, ones_sb, id_sb, masks, qt, kt, vt,
             aT_l, aT_f, aT_l7, aT_f7, wo, outT):
    with (
        tc.tile_pool(name="p2m", bufs=1) as mpool,
        tc.tile_pool(name="p2pt", bufs=3) as ptpool,
        tc.tile_pool(name="p2st", bufs=3) as stage,
        tc.tile_pool(name="p2pss", bufs=2, space="PSUM") as pss,
        tc.tile_pool(name="p2psa", bufs=2, space="PSUM") as psa,
        tc.tile_pool(name="p2psl", bufs=1, space="PSUM") as psl,
        tc.tile_pool(name="p3w", bufs=1) as wopool,
        tc.tile_pool(name="p3a", bufs=12) as apool,
        tc.tile_pool(name="p3st", bufs=2) as stage3,
        tc.tile_pool(name="p3ps", bufs=1, space="PSUM") as pso,
    ):
        masks_sb = mpool.tile([128, 4 * SB], BF16)
        for p in range(4):
            nc.sync.dma_start(out=masks_sb[:, p * SB:(p + 1) * SB],
                              in_=masks[p])
        wot = []
        for dml in range(HL):
            t = wopool.tile([128, DM], BF16, tag=f"wo{dml}")
            nc.sync.dma_start(out=t[:], in_=wo[dml])
            wot.append(t)

        def atg_load(b, gate=None):
            """Prefetch the gathered A^T of block b into SBUF (4 tiles).

            `gate` is an SBUF tile from the flash stream: a tiny copy of it
            into each destination tile makes the load (and everything that
            consumes it) order after that point of the flash stream in BOTH
            the scheduler's model and on hardware — without it the scheduler
            hoists the dependent out-projection matmuls into earlier exp-wait
            bubbles, where they stall the in-order PE queue on the (slow)
            AllGather."""
            atg = []
            for r in range(GROUPS):
                t = apool.tile([128, HL * SB], BF16, tag="atf")
                if gate is not None:
                    nc.vector.tensor_copy(t[:, :16], gate[:, :16])
                if b < NSB - 1:
                    nc.sync.dma_start(
                        out=t[:], in_=aT_f[b][r * 128:(r + 1) * 128, :])
                else:
                    for h in range(HL):
                        nc.gpsimd.dma_start(
                            out=t[:, h * SB:(h + 1) * SB],
                            in_=aT_f7[h][r * 128:(r + 1) * 128, :])
                atg.append(t)
            return atg

        def out_steps(b, atg):
            """Generator of phase-3 emission steps for block b."""
            for dml in range(HL):
                ps = pso.tile([128, SB], F32, tag="o")
                for i in range(ECH):
                    h, r = divmod(i, GROUPS)   # head-major: head 3 last
                    nc.tensor.matmul(
                        ps[:],
                        wot[dml][:, (r * HL + h) * 128:(r * HL + h + 1) * 128],
                        atg[r][:, h * SB:(h + 1) * SB],
                        start=(i == 0), stop=(i == ECH - 1))
                    yield
                osb = stage3.tile([128, SB], F32, tag="osb")
                nc.vector.tensor_copy(osb[:], ps[:])
                nc.sync.dma_start(
                    out=outT[dml * 128:(dml + 1) * 128,
                             b * SB:(b + 1) * SB],
                    in_=osb[:])
                yield

        atgs = {}
        pending = []

        def drain(n):
            if len(pending) > 1:
                n += 1
            done = 0
            while pending and done < n:
                try:
                    next(pending[0])
                    done += 1
                except StopIteration:
                    pending.pop(0)

        last = NSB - 1
        for j in range(NSB):
            for h in range(HL):
                at = _flash_block(nc, ones_sb, id_sb, masks_sb, qt, kt, vt,
                                  ptpool, stage, pss, psa, psl,
                                  aT_l, aT_l7, j, h, drain)
                if j == last:
                    nc.gpsimd.collective_compute(
                        "AllGather", mybir.AluOpType.bypass,
                        replica_groups=REPLICA_GROUPS,
                        ins=[aT_l7[h][:]], outs=[aT_f7[h][:]])
                    if h == 0:
                        atgs[5] = atg_load(5, gate=at)
                    elif h == 1:
                        pending.append(out_steps(5, atgs[5]))
                        atgs[6] = atg_load(6, gate=at)
                # interleave block j-3 from flash(j) h0 onward; its gather
                # was prefetched (gated) at the end of flash(j-1)
                if h == 0 and 3 <= j:
                    pending.append(out_steps(j - 3, atgs[j - 3]))
            if j < last:
                nc.gpsimd.collective_compute(
                    "AllGather", mybir.AluOpType.bypass,
                    replica_groups=REPLICA_GROUPS,
                    ins=[aT_l[j][:]], outs=[aT_f[j][:]])
            if 2 <= j <= 6:
                # gated prefetch at the end of flash(j): block j-2's
                # AllGather completed during this flash block, and the gate
                # keeps the scheduler from hoisting the loads (and the
                # out-matmuls behind them) into earlier exp-wait bubbles
                atgs[j - 2] = atg_load(j - 2, gate=at)
        # tail: remaining interleave leftovers, then blocks 6 and 7
        drain(10**9)
        for _ in out_steps(6, atgs[6]):
            pass
        atgs[last] = atg_load(last)
        for _ in out_steps(last, atgs[last]):
            pass


def _flash_block(nc, ones_sb, id_sb, masks_sb, qt, kt, vt, ptpool, stage,
                 pss, psa, psl, aT_l, aT_l7, j, h, drain):
    ps_a = psa.tile([128, SB], F32, tag="a")
    ps_l = psl.tile([128, SB], F32, tag="l")
    nk = 4 * j + 4
    npair = nk // 2
    for pr in range(npair):
        # two k-chunks share one 1024-wide PSUM tile so the exp runs once
        # per pair (amortizes ACT fixed overhead)
        ps_s = pss.tile([128, 2 * SB], F32, tag="s")
        pt = ptpool.tile([128, 2 * SB], BF16, tag="pt")
        offs = []
        for half in range(2):
            kc = 2 * pr + half
            # causal: columns below p*128 of a diagonal chunk are fully
            # masked; skip them in every consumer
            off = max(0, (kc - 4 * j) * 128) if kc >= 4 * j else 0
            offs.append(off)
            sl = slice(half * SB + off, (half + 1) * SB)
            diag = kc >= 4 * j
            nc.tensor.matmul(ps_s[:, sl],
                             kt[h][:, kc * 128:(kc + 1) * 128],
                             qt[h][:, j * SB + off:(j + 1) * SB],
                             start=True, stop=not diag)
            if diag:
                # add the causal mask on the PE: ps_s += I^T @ mask
                p = kc - 4 * j
                nc.tensor.matmul(
                    ps_s[:, sl], id_sb[:],
                    masks_sb[:, p * SB + off:(p + 1) * SB],
                    start=False, stop=True)
        nc.scalar.activation(pt[:, offs[0]:], ps_s[:, offs[0]:],
                             AF.Exp, scale=SCALE)
        for half in range(2):
            kc = 2 * pr + half
            off = offs[half]
            sl = slice(half * SB + off, (half + 1) * SB)
            osl = slice(off, SB)
            nc.tensor.matmul(ps_l[:, osl], ones_sb[:], pt[:, sl],
                             start=(kc == 0), stop=(kc == nk - 1))
            nc.tensor.matmul(ps_a[:, osl],
                             vt[:, h * S + kc * 128:h * S + (kc + 1) * 128],
                             pt[:, sl],
                             start=(kc == 0), stop=(kc == nk - 1))
        drain(2)
    lnl = stage.tile([128, SB], F32, tag="lnl")
    nc.scalar.activation(lnl[:], ps_l[:], AF.Ln)
    linv = stage.tile([128, SB], F32, tag="linv")
    nc.scalar.activation(linv[:], lnl[:], AF.Exp, scale=-1.0)
    at = stage.tile([128, SB], BF16, tag="at")
    nc.vector.tensor_mul(at[:], ps_a[:], linv[:])
    if j < NSB - 1:
        nc.sync.dma_start(out=aT_l[j][:, h * SB:(h + 1) * SB], in_=at[:])
    else:
        nc.sync.dma_start(out=aT_l7[h][:], in_=at[:])
    return at


def _host_prep(x, Wq, Wk, Wv, Wo):
    import ml_dtypes
    bf16 = ml_dtypes.bfloat16
    perm = np.concatenate([np.arange(0, DH, 2), np.arange(1, DH, 2)])  # evens then odds
    rowperm = np.concatenate([h * DH + perm for h in range(HL)])

    def tile_w(Wg):  # (EL, DM) -> (128, ECH*EL): [p, ec*EL+m] = Wg[m, ec*128+p]
        return np.ascontiguousarray(
            Wg.reshape(EL, ECH, 128).transpose(2, 1, 0).reshape(128, ECH * EL)
        ).astype(bf16)

    inv_freq = (1.0 / (10000.0 ** (np.arange(0, DH, 2) / DH))).astype(np.float64)
    pos = np.arange(S, dtype=np.float64)
    freqs = np.outer(inv_freq, pos)  # (64, S)
    cosT = np.concatenate([np.cos(freqs), np.cos(freqs)], 0).astype(bf16)
    sinT = np.concatenate([-np.sin(freqs), np.sin(freqs)], 0).astype(bf16)

    swap = np.zeros((128, 128), np.float32)
    for m in range(128):
        swap[(m + 64) % 128, m] = 1.0
    onesW = np.ones((128, 128), np.float32)
    masks = np.zeros((4, 128, SB), np.float32)
    ki = np.arange(128)[:, None]
    qi = np.arange(SB)[None, :]
    for p in range(4):
        masks[p] = np.where(qi >= ki + p * 128, 0.0, MASK_NEG)

    in_maps = []
    for c in range(N_CORES):
        b, g = divmod(c, GROUPS)
        sl = slice(g * EL, (g + 1) * EL)
        # wo[dml, p, ec*128+m] = Wo[g*EL + dml*128 + m, ec*128 + p]
        wo_t = np.ascontiguousarray(
            Wo[sl].reshape(HL, 128, ECH, 128).transpose(0, 3, 2, 1)
            .reshape(HL, 128, DM)).astype(bf16)
        in_maps.append({
            "xT": np.ascontiguousarray(x[b].T).astype(bf16),
            "wq": tile_w(Wq[sl][rowperm]),
            "wk": tile_w(Wk[sl][rowperm]),
            "wv": tile_w(Wv[sl]),
            "wo": wo_t,
            "cosT": cosT,
            "sinT": sinT,
            "swapM": swap.astype(bf16),
            "onesW": onesW.astype(bf16),
            "idM": np.eye(128, dtype=np.float32).astype(bf16),
            "masks": masks.astype(bf16),
        })
    return in_maps


def kernel(x, Wq, Wk, Wv, Wo):
    in_maps = _host_prep(x, Wq, Wk, Wv, Wo)
    nc = build_nc()
    res = bass2jax.run_bass_via_pjrt(nc, in_maps, n_cores=N_CORES)
    out = np.empty((B, S, DM), np.float32)
    for c in range(N_CORES):
        b, g = divmod(c, GROUPS)
        out[b, :, g * EL:(g + 1) * EL] = res[c]["outT"].T
    return out


if __name__ == "__main__":
    rng = np.random.default_rng(0)
    x = rng.standard_normal((B, S, DM)).astype(np.float32)
    Wq = (rng.standard_normal((H * DH, DM)) * 0.02).astype(np.float32)
    Wk = (rng.standard_normal((H * DH, DM)) * 0.02).astype(np.float32)
    Wv = (rng.standard_normal((H * DH, DM)) * 0.02).astype(np.float32)
    Wo = (rng.standard_normal((DM, H * DH)) * 0.02).astype(np.float32)
    out = kernel(x, Wq, Wk, Wv, Wo)
    print(out.shape, out.dtype)
